# revision 4
# baseline (speedup 1.0000x reference)
"""Correlation cost-volume (SpatialCorrelationSampler k=1, patch=9) + leaky ReLU.

Full inputs: feat1, feat2 [16, 256, 96, 160] f32.  Output [16, 81, 96, 160] f32.
corr[b, 9*i+j, y, x] = leaky_relu(sum_c f1[b,c,y,x] * f2[b,c,y+i-4,x+j-4], 0.1)

Strategy (8 NeuronCores, data-parallel over batch, 2 images/core):
  - per (image, 80-col half, row y): Gram-band matmuls on TensorE in bf16:
      lhsT = f1[c_chunk, y, x0:x0+80]            [K=128, M=80]
      rhs  = f2pad[c_chunk, y-4..y+4, x0-4..x0+83] -> [K=128, 9*88] contiguous
    2 C-chunks accumulate in PSUM (dys 0..4 -> psum[:,0:440], dys 5..8 -> [512:864],
    each region inside one PSUM bank).
  - ScalarE evicts PSUM with Prelu(alpha=0.1) into S[80, 792] in *interleaved*
    layout col = 9*x' + dy.  Then the 81 band values of partition m are the
    contiguous run S[m, 9m : 9m+81] (ordered k = 9*dx + dy).
  - One skewed DMA per row (step = rowlen+9 across partitions, 81-contig runs)
    writes the band straight to DRAM; host reorders the 81 channels + x tiles.
"""

import numpy as np

import bass_rust
import concourse.bacc as bacc
import concourse.bass as bass
import concourse.mybir as mybir
import concourse.tile as tile
from concourse.bass_utils import run_bass_kernel_spmd

B, C, H, W = 16, 256, 96, 160
NCORES = 8
NB = B // NCORES          # images per core
WH = 80                   # column-half width (matmul M)
WPAD = WH + 8             # rhs window width
HPAD = H + 8              # zero-padded rows
NPATCH = 81


def build_nc(leaky: bool = True, units: list | None = None) -> bass.Bass:
    nc = bacc.Bacc()
    f1_ext = nc.declare_dram_parameter(
        "feat1", [NB, C, H, W], mybir.dt.float32, isOutput=False)
    f2_ext = nc.declare_dram_parameter(
        "feat2", [NB, C, H, W], mybir.dt.float32, isOutput=False)
    out_ext = nc.declare_dram_parameter(
        "out", [NB, 2, H, WH, NPATCH], mybir.dt.float32, isOutput=True)

    act_fn = (mybir.ActivationFunctionType.Prelu if leaky
              else mybir.ActivationFunctionType.Relu)
    if units is None:
        units = [(b, h) for b in range(NB) for h in range(2)]

    SROWS = 24            # rows per load strip
    NSTRIP = H // SROWS

    with tile.TileContext(nc) as tc:
        with (
            tc.tile_pool(name="feat", bufs=2) as featp,
            tc.tile_pool(name="stage", bufs=3) as stagep,
            tc.tile_pool(name="spool", bufs=3) as spool,
            tc.tile_pool(name="psum", bufs=3, space="PSUM") as psump,
        ):
            for (b, h) in units:
                x0 = WH * h
                f1c = [featp.tile([128, H * WH], mybir.dt.bfloat16, tag=f"f1c{c}",
                                  name=f"f1c{c}_{b}_{h}")
                       for c in range(2)]
                f2c = [featp.tile([128, HPAD * WPAD], mybir.dt.bfloat16, tag=f"f2c{c}",
                                  name=f"f2c{c}_{b}_{h}")
                       for c in range(2)]
                for c in range(2):
                    nc.gpsimd.memset(f2c[c][:, :], 0.0)

                # f1: load f32 strips, cast to bf16 (row-major [H, 80] per chunk)
                for c in range(2):
                    for s in range(NSTRIP):
                        st = stagep.tile([128, SROWS * 84], mybir.dt.float32,
                                         tag="stage", name=f"stage_{b}_{h}_{c}_{s}")
                        nc.sync.dma_start(
                            st[:, :SROWS * WH],
                            f1_ext[b, 128 * c:128 * (c + 1),
                                   SROWS * s:SROWS * (s + 1), x0:x0 + WH])
                        nc.vector.tensor_copy(
                            f1c[c][:, s * SROWS * WH:(s + 1) * SROWS * WH],
                            st[:, :SROWS * WH])

                # f2: load valid 84-col window, cast into padded [HPAD, 88] layout
                xs_lo = max(0, x0 - 4)
                dcol = 4 if h == 0 else 0
                for c in range(2):
                    for s in range(NSTRIP):
                        st = stagep.tile([128, SROWS * 84], mybir.dt.float32,
                                         tag="stage", name=f"stage_{b}_{h}_{c}_{s}")
                        nc.sync.dma_start(
                            st[:, :],
                            f2_ext[b, 128 * c:128 * (c + 1),
                                   SROWS * s:SROWS * (s + 1), xs_lo:xs_lo + 84])
                        f2v = f2c[c][:, :].rearrange("p (r w) -> p r w", w=WPAD)
                        nc.vector.tensor_copy(
                            f2v[:, 4 + SROWS * s:4 + SROWS * (s + 1),
                                dcol:dcol + 84],
                            st[:, :].rearrange("p (r w) -> p r w", w=84))

                for y in range(H):
                    ps = psump.tile([WH, 1024], mybir.dt.float32, tag="ps", name=f"ps_{b}_{h}_{y}")
                    S = spool.tile([WH, 9 * WPAD], mybir.dt.float32, tag="S", name=f"S_{b}_{h}_{y}")
                    r0 = y * WPAD
                    lhs = [f1c[c][:, y * WH:(y + 1) * WH] for c in range(2)]
                    nc.tensor.matmul(ps[0:WH, 0:440], lhs[0],
                                     f2c[0][:, r0:r0 + 440],
                                     start=True, stop=False)
                    nc.tensor.matmul(ps[0:WH, 512:864], lhs[0],
                                     f2c[0][:, r0 + 440:r0 + 792],
                                     start=True, stop=False)
                    nc.tensor.matmul(ps[0:WH, 0:440], lhs[1],
                                     f2c[1][:, r0:r0 + 440],
                                     start=False, stop=True)
                    nc.tensor.matmul(ps[0:WH, 512:864], lhs[1],
                                     f2c[1][:, r0 + 440:r0 + 792],
                                     start=False, stop=True)

                    # evict + leaky relu into interleaved S: col = 9*x' + dy
                    Sb = S[:, :]
                    RL = Sb.ap[0][0]
                    dst1 = bass_rust.AP(Sb.tensor, Sb.offset,
                                        [[RL, WH], [1, 5], [9, WPAD]])
                    nc.scalar.activation(
                        dst1,
                        ps[0:WH, 0:440].rearrange("p (d x) -> p d x", d=5),
                        act_fn, alpha=0.1)
                    dst2 = bass_rust.AP(Sb.tensor, Sb.offset + 5,
                                        [[RL, WH], [1, 4], [9, WPAD]])
                    nc.scalar.activation(
                        dst2,
                        ps[0:WH, 512:864].rearrange("p (d x) -> p d x", d=4),
                        act_fn, alpha=0.1)

                    # skewed band DMA: partition m reads S[m, 9m : 9m+81]
                    diag = bass_rust.AP(Sb.tensor, Sb.offset,
                                        [[RL + 9, WH], [1, NPATCH]])
                    nc.scalar.dma_start(out_ext[b, h, y], diag)
    nc.finalize()
    return nc


_CACHE: dict = {}


def _get_nc() -> bass.Bass:
    if "nc" not in _CACHE:
        _CACHE["nc"] = build_nc(leaky=True)
    return _CACHE["nc"]


# channel reorder: device emits k = 9*dx + dy, reference wants d = 9*dy + dx
_PERM = np.arange(NPATCH).reshape(9, 9).T.reshape(-1)


def _assemble(core_outs: list) -> np.ndarray:
    full = np.empty((B, NPATCH, H, W), dtype=np.float32)
    for i, a in enumerate(core_outs):
        a = np.asarray(a).reshape(NB, 2, H, WH, NPATCH)[..., _PERM]
        full[NB * i:NB * (i + 1)] = (
            a.transpose(0, 4, 2, 1, 3).reshape(NB, NPATCH, H, W))
    return full


def kernel(feat1: np.ndarray, feat2: np.ndarray, **_ignored) -> np.ndarray:
    feat1 = np.ascontiguousarray(np.asarray(feat1), dtype=np.float32)
    feat2 = np.ascontiguousarray(np.asarray(feat2), dtype=np.float32)
    nc = _get_nc()
    in_maps = [
        {"feat1": feat1[NB * i:NB * (i + 1)], "feat2": feat2[NB * i:NB * (i + 1)]}
        for i in range(NCORES)
    ]
    res = run_bass_kernel_spmd(nc, in_maps, list(range(NCORES)))
    return _assemble([res.results[i]["out"] for i in range(NCORES)])
